# revision 24
# baseline (speedup 1.0000x reference)
"""Trainium2 Bass kernel for nn_CP_Attention_Action (dense transformer block with
CP-factored low-rank corrections).

Data-parallel over batch B=8 -> one batch per NeuronCore, no collectives.

The CP branch is affine in its input, so it is folded into the dense weights on
the host (f64): W_qkv_eff = qkv_w.T + U.T @ F_f @ V.T blocks, with the q/k
biases applied at evacuation and the v bias folded through softmax (weights sum
to 1) into the proj bias. The device then runs a plain attention block:

  qkT (o,n) via stationary W tiles; v in natural (m,d) layout with a ones
  column so the softmax denominator falls out of attn@v; logits via 64x128
  row-group-paired matmuls (hl0/hl1 issued adjacently -> both stream on PE
  concurrently); exp on ScalarE in 1536-wide batches (all-ones mask -> uniform
  zero bias); denominator: copy+reciprocal on DVE + partition_broadcast on
  GpSimd; attn@v of pair p runs as PE filler during pair p+1; pair 3 uses an
  ncx-major st layout so its ncx0 attn@v half completes mid-pair and only the
  ncx1 half + proj remain in the tail. Host pre-arranges all inputs into
  partition-major layout for fat contiguous DMA descriptors on 4 queues.
"""

import os
import numpy as np
import ml_dtypes

from concourse import bacc
import concourse.mybir as mybir
import concourse.tile as tile
from concourse.bass_utils import run_bass_kernel_spmd

B, N, DIM = 8, 1024, 512
H, D = 8, 64
SCALE = D ** -0.5
NCORES = 8
NC_CH = 2          # n chunks of 512
NT = N // 128      # 8 key tiles
CT = DIM // 128    # 4 c-tiles
OT = 8             # q,k o-tiles
PAIRW = 2 * N      # free elems per key-tile slot in st (2 heads x 1024 q)
STW = NT * PAIRW   # st elems per pair (16384)
LPW = 1536         # activation batch width (3 PSUM banks)
F32 = mybir.dt.float32
BF = mybir.dt.bfloat16
AF = mybir.ActivationFunctionType
bf16 = ml_dtypes.bfloat16

_CACHE = {}


QK_ORDER = (0, 4, 1, 5, 2, 6, 3, 7)


def _stpos(pair, mt, hl, ncx):
    # ncx-major: the first half of each pair's stream only needs the first
    # n-half of q, so pair 0 can start before the second xT half lands
    return ncx * (NT * 1024) + mt * 1024 + hl * 512


def _slots(pair):
    # (mt, hl, ncx) in ascending stream-position order, hl adjacent
    out = []
    for ncx in range(NC_CH):
        for mt in range(NT):
            for hl in range(2):
                out.append((mt, hl, ncx))
    return out


def _build(mask_ones: bool):
    nc = bacc.Bacc()

    xA_e = nc.declare_dram_parameter("xA", [128, CT, 512], BF, isOutput=False)
    xB_e = nc.declare_dram_parameter("xB", [128, CT, 512], BF, isOutput=False)
    wqk1_e = nc.declare_dram_parameter("wqk1", [128, CT, 256], BF, isOutput=False)
    wqk2_e = nc.declare_dram_parameter("wqk2", [128, CT, 768], BF, isOutput=False)
    wvB_e = nc.declare_dram_parameter("wvB", [128, CT, DIM], BF, isOutput=False)
    wproj_e = nc.declare_dram_parameter("wproj", [128, CT, DIM], BF, isOutput=False)
    fcon_e = nc.declare_dram_parameter("fcon", [128, 24], F32, isOutput=False)
    out_e = nc.declare_dram_parameter("out", [128, NC_CH, CT, 512], BF, isOutput=True)

    # fcon layout (f32 columns): 0:8 qk bias per ot | 8:12 proj bias | 12:20 maskb
    with tile.TileContext(nc) as tc:
        with tc.tile_pool(name="consts", bufs=1) as consts, \
             tc.tile_pool(name="qkpool", bufs=1) as qkpool, \
             tc.tile_pool(name="stpool", bufs=2) as stpool, \
             tc.tile_pool(name="work", bufs=1) as work, \
             tc.tile_pool(name="dnpool", bufs=4) as dnpool, \
             tc.tile_pool(name="bcpool", bufs=2) as bcpool, \
             tc.tile_pool(name="outp", bufs=1) as outp, \
             tc.tile_pool(name="popool", bufs=1) as popool:

            # ---------- inputs: 4 DMA queues, partition-major layouts ----------
            warm = consts.tile([128, 512], BF)
            nc.vector.memset(warm[:], 0.0)
            scr = consts.tile([1, 16], BF)
            scr2 = consts.tile([1, 16], BF)
            nc.vector.memset(scr[:], 0.0)
            xh = [consts.tile([128, CT, 512], BF, name=f"xh{i}") for i in range(2)]
            fcon = consts.tile([128, 24], F32)
            wqk1 = consts.tile([128, CT, 256], BF)
            wqk2 = consts.tile([128, CT, 768], BF)
            nc.sync.dma_start(xh[0][:], xA_e[:, :, :])
            nc.sync.dma_start(xh[1][:], xB_e[:, :, :])
            nc.gpsimd.dma_start(wqk1[:], wqk1_e[:, :, :])
            nc.gpsimd.dma_start(wqk2[:], wqk2_e[:, :, :])
            nc.scalar.dma_start(fcon[:], fcon_e[:, :])
            wvB = consts.tile([128, CT, DIM], BF)
            nc.scalar.dma_start(wvB[:], wvB_e[:, :, :])
            wproj = consts.tile([128, CT, DIM], BF)
            nc.scalar.dma_start(wproj[:], wproj_e[:, :, :])

            def qkbias(ot):
                return fcon[:, ot:ot + 1]

            def pbias(ot):
                return fcon[:, 8 + ot:9 + ot]

            def maskb(mt):
                return fcon[:, 12 + mt:13 + mt]

            outT = outp.tile([128, CT, N], BF)
            v_sb = work.tile([128, NT, H, D + 1], BF)
            qk = qkpool.tile([128, OT, N], BF)
            nc.vector.memset(v_sb[:, :, :, D:D + 1], 1.0)
            # early exp-table load on a scratch tile (overlaps the DMA wait)
            nc.scalar.activation(scr2[:], scr[:], AF.Exp, bias=0.0, scale=1.0)

            # ---------- warm-up matmuls (HAM un-throttle during DMA wait) ----------
            wscr = consts.tile([1, 16], F32)
            with tc.tile_pool(name="wps", bufs=1, space="PSUM") as wps:
                wp = wps.tile([128, 512], F32)
                for i in range(9):
                    nc.tensor.matmul(wp[:, :], lhsT=warm[:, 0:128], rhs=warm[:, :],
                                     start=(i == 0), stop=(i == 8))
                # reader so later pools' bank reuse orders after the warm-up
                nc.vector.tensor_copy(wscr[0:1, :], wp[0:1, 0:16])

            with tc.tile_pool(name="lpp", bufs=2, space="PSUM") as lpp, \
                 tc.tile_pool(name="avp", bufs=2, space="PSUM") as avp:

                def qk_items(ot):
                    items = []
                    for ncx in range(NC_CH):
                        box = {}

                        def mk(ct, box=box, ncx=ncx, ot=ot):
                            def mm():
                                if "t" not in box:
                                    box["t"] = avp.tile([128, 512], F32, tag="av",
                                                        name=f"qp{ot}_{ncx}")
                                j = QK_ORDER.index(ot)
                                w = wqk1[:, ct, j * 128:(j + 1) * 128] if j < 2 \
                                    else wqk2[:, ct, (j - 2) * 128:(j - 1) * 128]
                                nc.tensor.matmul(
                                    box["t"][:, :], lhsT=w,
                                    rhs=xh[ncx][:, ct, :],
                                    start=(ct == 0), stop=(ct == CT - 1))
                            return mm

                        def ev(box=box, ncx=ncx, ot=ot):
                            nc.vector.tensor_scalar_add(
                                qk[:, ot, ncx * 512:(ncx + 1) * 512],
                                box["t"][:, :], qkbias(ot))
                        for ct in range(CT):
                            items.append(mk(ct))
                        items.append(ev)
                    return items

                # qk pair-0 weights first so pair 0 can start immediately
                for it in qk_items(0) + qk_items(4):
                    it()

                def v_items():
                    items = []
                    for mt in range(NT):
                        box = {}

                        def mk(ct, box=box, mt=mt):
                            def mm():
                                if "t" not in box:
                                    box["t"] = avp.tile([128, 512], F32, tag="av",
                                                        name=f"vp{mt}")
                                nc.tensor.matmul(
                                    box["t"][:, :],
                                    lhsT=xh[mt // 4][:, ct, (mt % 4) * 128:
                                                     (mt % 4) * 128 + 128],
                                    rhs=wvB[:, ct, :],
                                    start=(ct == 0), stop=(ct == CT - 1))
                            return mm

                        def ev(box=box, mt=mt):
                            nc.vector.tensor_copy(
                                v_sb[:, mt, :, 0:D],
                                box["t"][:, :].rearrange("p (h d) -> p h d", h=H))
                        for ct in range(CT):
                            items.append(mk(ct))
                        items.append(ev)
                    return items

                # ---------- attention ----------
                st_t = [None, None]

                def av_group(pair, hl, ncx, tail=False):
                    # attn@v accumulation + denominator chain for one head/ncx
                    items = []
                    stp = st_t[pair % 2]
                    h = 2 * pair + hl
                    box = {}

                    def mk(mt, box=box, hl=hl, ncx=ncx, h=h, pair=pair):
                        def mm():
                            if "t" not in box:
                                box["t"] = avp.tile([128, 512], F32, tag="av",
                                                    name=f"ap{h}_{ncx}")
                            p0 = _stpos(pair, mt, hl, ncx)
                            nc.tensor.matmul(
                                box["t"][0:D + 1, :],
                                lhsT=v_sb[:, mt, h, :],
                                rhs=stp[:, p0:p0 + 512],
                                start=(mt == 0), stop=(mt == NT - 1))
                        return mm

                    def chain(box=box, hl=hl, ncx=ncx, pair=pair, h=h, tail=tail):
                        # custom DVE ops drop the input base partition: copy the
                        # den row to partition 0 before the reciprocal
                        dnr = dnpool.tile([1, 512], F32, tag="dnr",
                                          name=f"dnr{h}_{ncx}")
                        if tail:   # ScalarE is idle after its last exp
                            nc.scalar.copy(dnr[0:1, :], box["t"][D:D + 1, :])
                        else:
                            nc.vector.tensor_copy(dnr[0:1, :], box["t"][D:D + 1, :])
                        dn = dnpool.tile([1, 512], F32, tag="dn",
                                         name=f"dn{h}_{ncx}")
                        nc.vector.reciprocal_approx_fast(dn[0:1, :], dnr[0:1, :])
                        bc = bcpool.tile([D, 512], F32, tag="bc",
                                         name=f"bc{h}_{ncx}")
                        nc.gpsimd.partition_broadcast(bc[:, :], dn[0:1, :])
                        nc.vector.tensor_mul(
                            outT[hl * 64:hl * 64 + 64, pair,
                                 ncx * 512:(ncx + 1) * 512],
                            box["t"][0:D, :], bc[:, :])
                    for mt in range(NT):
                        items.append(mk(mt))
                    items.append(chain)
                    return items

                po_sb = popool.tile([128, NC_CH, CT, 512], BF)

                def proj_items(ncx):
                    items = []
                    for ot in range(CT):
                        box = {}

                        def mk(kt, box=box, ot=ot, ncx=ncx):
                            def mm():
                                if "t" not in box:
                                    if ncx == 1 and ot >= 2:
                                        # borrow the (idle in tail) av ring so
                                        # all 4 groups can pre-run kt0..2
                                        box["t"] = avp.tile([128, 512], F32,
                                                            tag="av",
                                                            name=f"pq{ot}")
                                    else:
                                        box["t"] = lpp.tile([128, LPW], F32,
                                                            tag="lp",
                                                            name=f"pp{ot % 2}")
                                nc.tensor.matmul(
                                    box["t"][:, 0:512],
                                    lhsT=wproj[:, kt, ot * 128:(ot + 1) * 128],
                                    rhs=outT[:, kt, ncx * 512:(ncx + 1) * 512],
                                    start=(kt == 0), stop=(kt == CT - 1))
                            return mm

                        def ev(box=box, ot=ot, ncx=ncx):
                            if ncx == 1 and ot % 2 == 0:  # tail: split engines
                                nc.scalar.add(po_sb[:, ncx, ot, :],
                                              box["t"][:, 0:512], pbias(ot))
                            else:
                                nc.vector.tensor_scalar_add(
                                    po_sb[:, ncx, ot, :], box["t"][:, 0:512],
                                    pbias(ot))
                        for kt in range(CT):
                            items.append(mk(kt))
                        items.append(ev)

                    def dma(ncx=ncx):
                        deng = nc.sync if ncx == 0 else nc.gpsimd
                        deng.dma_start(out_e[:, ncx, :, :], po_sb[:, ncx, :, :])
                    items.append(dma)
                    return items

                filler = []
                fi = 0

                def consume(k):
                    nonlocal fi
                    e = min(fi + k, len(filler))
                    while fi < e:
                        filler[fi]()
                        fi += 1

                for pair in range(4):
                    stp = stpool.tile([128, STW], BF, tag="st", name=f"st{pair % 2}")
                    st_t[pair % 2] = stp
                    if pair == 0:
                        # v + next pair's qk weights; later pairs: av(p-1) + qk
                        filler = v_items() + qk_items(1) + qk_items(5)
                        fi = 0
                    elif pair == 1:
                        filler = filler[fi:] + qk_items(2) + qk_items(6)
                        fi = 0
                    elif pair == 2:
                        filler = filler[fi:] + qk_items(3) + qk_items(7)
                        fi = 0
                    elif pair == 3:
                        # phase A: av(2); phase B (from slot 16, after the ncx0
                        # chunks' ACTs are emitted): av(3) ncx0 + proj(ncx0)
                        filler = filler[fi:]
                        fi = 0
                        fillerB = av_group(3, 0, 0) + av_group(3, 1, 0) + \
                            proj_items(0)
                    npts = len(_slots(pair)) // 2
                    if pair == 3:
                        npts = npts // 2   # phase A gets slots 0..15 only
                    per = max(1, (len(filler) - fi + npts - 1) // npts)

                    lpt = {}
                    acted = 0
                    for si, (mt, hl, ncx) in enumerate(_slots(pair)):
                        pos = _stpos(pair, mt, hl, ncx)
                        ti = pos // LPW
                        off = pos - ti * LPW
                        if ti not in lpt:
                            lpt[ti] = lpp.tile([128, LPW], F32, tag="lp",
                                               name=f"lp{pair}_{ti % 2}")
                        pb = hl * 64
                        nc.tensor.matmul(
                            lpt[ti][:, off:off + 512],
                            lhsT=qk[pb:pb + 64, 4 + pair, mt * 128:(mt + 1) * 128],
                            rhs=qk[pb:pb + 64, pair, ncx * 512:(ncx + 1) * 512],
                            start=True, stop=True, tile_position=(pb, 0))
                        if not mask_ones:
                            nc.scalar.activation(
                                stp[:, pos:pos + 512], lpt[ti][:, off:off + 512],
                                AF.Exp, bias=maskb(mt), scale=SCALE)
                            if off + 512 == LPW or pos + 512 == STW:
                                del lpt[ti]
                        else:
                            while acted * LPW + LPW <= pos + 512 or \
                                    pos + 512 == STW:
                                w = min(LPW, STW - acted * LPW)
                                nc.scalar.activation(
                                    stp[:, acted * LPW:acted * LPW + w],
                                    lpt[acted][:, 0:w], AF.Exp,
                                    bias=0.0, scale=SCALE)
                                del lpt[acted]
                                acted += 1
                                if acted * LPW >= STW:
                                    break
                        if pair == 3 and si == 16:
                            consume(len(filler))   # finish phase A
                            filler = fillerB
                            fi = 0
                            per = max(1, (len(filler) + npts - 1) // npts)
                        if si % 2 == 1:
                            consume(per)   # only between hl pairs (PE row pairing)

                    consume(len(filler))
                    if pair < 3:
                        filler = []
                        for ncx in range(NC_CH):
                            for hl in range(2):
                                filler += av_group(pair, hl, ncx)
                        fi = 0

                # ---------- tail: av(3) ncx1 + proj(ncx1) ----------
                avt = av_group(3, 0, 1, tail=True) + av_group(3, 1, 1, tail=True)
                pj = proj_items(1)
                # kt0..2 of all four groups run early (they read pairs 0-2
                # outT only); every kt3 matmul is emitted after both chains.
                # lpp-ring groups (ot0/1) interleave with the first av group;
                # avp-ring groups (ot2/3) only after ap7's tile is allocated.
                pre1 = pj[0:3] + pj[5:8]
                pre2 = pj[10:13] + pj[15:18]
                fin = pj[3:5] + pj[8:10] + pj[13:15] + pj[18:]
                t1 = t2 = 0
                for ai, it in enumerate(avt):
                    it()
                    if ai < 9:
                        if t1 < len(pre1):
                            pre1[t1]()
                            t1 += 1
                    else:
                        if t2 < len(pre2):
                            pre2[t2]()
                            t2 += 1
                while t1 < len(pre1):
                    pre1[t1]()
                    t1 += 1
                while t2 < len(pre2):
                    pre2[t2]()
                    t2 += 1
                for it in fin:
                    it()

    nc.compile()
    return nc


def _prep(inputs):
    x = np.asarray(inputs["x"])
    mask = np.asarray(inputs["mask"])
    qkv_w = np.asarray(inputs["qkv_w"], np.float64)
    CP_U_w = np.asarray(inputs["CP_U_w"], np.float64)
    CP_U_b = np.asarray(inputs["CP_U_b"], np.float64)
    CP_V_w = np.asarray(inputs["CP_V_w"], np.float64)
    CP_V_b = np.asarray(inputs["CP_V_b"], np.float64)
    CP_C = np.asarray(inputs["CP_C"], np.float64)
    CP_att = np.asarray(inputs["CP_attention"], np.float64)
    proj_w = np.asarray(inputs["proj_w"], np.float64)
    proj_b = np.asarray(inputs["proj_b"], np.float64)

    # fold the CP branch (affine in its input) into the dense weights
    F = np.einsum('ijr,rf->fij', CP_C, CP_att)          # (4, R, R)
    UT = CP_U_w.T                                        # (DIM, R)
    VT = CP_V_w.T                                        # (R, DIM)
    A = np.stack([UT @ F[f] @ VT for f in range(4)])     # (4, DIM, DIM)
    c = np.stack([CP_U_b @ F[f] @ VT + CP_V_b for f in range(4)])  # (4, DIM)

    Wqkv = qkv_w.T + np.concatenate([A[0], A[1], A[2]], axis=1)   # (DIM, 3*DIM)
    Wp = proj_w.T + A[3]                                          # (DIM, DIM)
    b_qk = np.concatenate([c[0], c[1]])                           # (2*DIM,)
    b_out = proj_b + c[3] + c[2] @ Wp                             # (DIM,)

    fcon = np.zeros((128, 24), np.float32)
    fcon[:, 0:8] = b_qk.reshape(OT, 128).T
    fcon[:, 8:12] = b_out.reshape(CT, 128).T

    def pmajor(w):
        # (DIM, W) -> (128, CT, W): partition p holds rows {p, 128+p, ...}
        return np.ascontiguousarray(
            w.reshape(CT, 128, w.shape[1]).transpose(1, 0, 2))

    mask_ones = bool(mask.all())
    wqk = pmajor(Wqkv[:, 0:2 * DIM])            # (128, CT, 1024)
    wqk_ord = np.concatenate([wqk[:, :, ot * 128:(ot + 1) * 128]
                              for ot in QK_ORDER], axis=2)
    com = {
        "wqk1": np.ascontiguousarray(wqk_ord[:, :, 0:256]).astype(bf16),
        "wqk2": np.ascontiguousarray(wqk_ord[:, :, 256:1024]).astype(bf16),
        "wvB": pmajor(Wqkv[:, 2 * DIM:3 * DIM]).astype(bf16),
        "wproj": pmajor(Wp).astype(bf16),
    }
    in_maps = []
    for b in range(B):
        m = dict(com)
        xp = pmajor(x[b].T.astype(np.float64))
        m["xA"] = np.ascontiguousarray(xp[:, :, 0:512]).astype(bf16)
        m["xB"] = np.ascontiguousarray(xp[:, :, 512:1024]).astype(bf16)
        fc = fcon.copy()
        if not mask_ones:
            mb = np.where(mask[b], 0.0, -1e30).astype(np.float32)
            fc[:, 12:20] = mb.reshape(NT, 128).T
        m["fcon"] = fc
        in_maps.append(m)
    return in_maps, mask_ones


LAST_EXEC_NS = None


def kernel(**inputs):
    global LAST_EXEC_NS
    in_maps, mask_ones = _prep(inputs)
    key = ("nc", mask_ones)
    if key not in _CACHE:
        _CACHE[key] = _build(mask_ones)
    nc = _CACHE[key]
    res = run_bass_kernel_spmd(nc, in_maps, core_ids=list(range(NCORES)))
    LAST_EXEC_NS = res.exec_time_ns
    outs = []
    for i in range(NCORES):
        ob = np.asarray(res.results[i]["out"], dtype=np.float32)  # (128,2,4,512)
        on = ob.transpose(2, 0, 1, 3).reshape(DIM, N)             # (o, n)
        outs.append(on.T.copy())
    return np.stack(outs).astype(np.float32)


# revision 25
# speedup vs baseline: 1.2108x; 1.2108x over previous
"""Trainium2 Bass kernel for nn_CP_Attention_Action (dense transformer block with
CP-factored low-rank corrections).

Data-parallel over batch B=8 -> one batch per NeuronCore, no collectives.

The CP branch is affine in its input, so it is folded into the dense weights on
the host (f64): W_qkv_eff = qkv_w.T + U.T @ F_f @ V.T blocks, with the q/k
biases applied at evacuation and the v bias folded through softmax (weights sum
to 1) into the proj bias. The device then runs a plain attention block:

  qkT (o,n) via stationary W tiles; v in natural (m,d) layout with a ones
  column so the softmax denominator falls out of attn@v; logits via 64x128
  row-group-paired matmuls (hl0/hl1 issued adjacently -> both stream on PE
  concurrently); exp on ScalarE in 1536-wide batches (all-ones mask -> uniform
  zero bias); denominator: copy+reciprocal on DVE + partition_broadcast on
  GpSimd; attn@v of pair p runs as PE filler during pair p+1; pair 3 uses an
  ncx-major st layout so its ncx0 attn@v half completes mid-pair and only the
  ncx1 half + proj remain in the tail. Host pre-arranges all inputs into
  partition-major layout for fat contiguous DMA descriptors on 4 queues.
"""

import os
import numpy as np
import ml_dtypes

from concourse import bacc
import concourse.mybir as mybir
import concourse.tile as tile
from concourse.bass_utils import run_bass_kernel_spmd

B, N, DIM = 8, 1024, 512
H, D = 8, 64
SCALE = D ** -0.5
NCORES = 8
NC_CH = 2          # n chunks of 512
NT = N // 128      # 8 key tiles
CT = DIM // 128    # 4 c-tiles
OT = 8             # q,k o-tiles
PAIRW = 2 * N      # free elems per key-tile slot in st (2 heads x 1024 q)
STW = NT * PAIRW   # st elems per pair (16384)
LPW = 1536         # activation batch width (3 PSUM banks)
F32 = mybir.dt.float32
BF = mybir.dt.bfloat16
AF = mybir.ActivationFunctionType
bf16 = ml_dtypes.bfloat16

_CACHE = {}


QK_ORDER = (0, 4, 1, 5, 2, 6, 3, 7)


def _stpos(pair, mt, hl, ncx):
    # ncx-major: the first half of each pair's stream only needs the first
    # n-half of q, so pair 0 can start before the second xT half lands
    return ncx * (NT * 1024) + mt * 1024 + hl * 512


def _slots(pair):
    # (mt, hl, ncx) in ascending stream-position order, hl adjacent
    out = []
    for ncx in range(NC_CH):
        for mt in range(NT):
            for hl in range(2):
                out.append((mt, hl, ncx))
    return out


def _build(mask_ones: bool):
    nc = bacc.Bacc()

    xA_e = nc.declare_dram_parameter("xA", [128, CT, 512], BF, isOutput=False)
    xB_e = nc.declare_dram_parameter("xB", [128, CT, 512], BF, isOutput=False)
    wqk1_e = nc.declare_dram_parameter("wqk1", [128, CT, 256], BF, isOutput=False)
    wqk2_e = nc.declare_dram_parameter("wqk2", [128, CT, 768], BF, isOutput=False)
    wvB_e = nc.declare_dram_parameter("wvB", [128, CT, DIM], BF, isOutput=False)
    wproj_e = nc.declare_dram_parameter("wproj", [128, CT, DIM], BF, isOutput=False)
    fcon_e = nc.declare_dram_parameter("fcon", [128, 24], F32, isOutput=False)
    out_e = nc.declare_dram_parameter("out", [128, NC_CH, CT, 512], BF, isOutput=True)

    # fcon layout (f32 columns): 0:8 qk bias per ot | 8:12 proj bias | 12:20 maskb
    with tile.TileContext(nc) as tc:
        with tc.tile_pool(name="consts", bufs=1) as consts, \
             tc.tile_pool(name="qkpool", bufs=1) as qkpool, \
             tc.tile_pool(name="stpool", bufs=2) as stpool, \
             tc.tile_pool(name="work", bufs=1) as work, \
             tc.tile_pool(name="dnpool", bufs=4) as dnpool, \
             tc.tile_pool(name="bcpool", bufs=2) as bcpool, \
             tc.tile_pool(name="outp", bufs=1) as outp, \
             tc.tile_pool(name="popool", bufs=1) as popool:

            # ---------- inputs: 4 DMA queues, partition-major layouts ----------
            warm = consts.tile([128, 512], BF)
            nc.vector.memset(warm[:], 0.0)
            scr = consts.tile([1, 16], BF)
            scr2 = consts.tile([1, 16], BF)
            nc.vector.memset(scr[:], 0.0)
            xh = [consts.tile([128, CT, 512], BF, name=f"xh{i}") for i in range(2)]
            fcon = consts.tile([128, 24], F32)
            wqk1 = consts.tile([128, CT, 256], BF)
            wqk2 = consts.tile([128, CT, 768], BF)
            nc.sync.dma_start(xh[0][:], xA_e[:, :, :])
            nc.sync.dma_start(xh[1][:], xB_e[:, :, :])
            nc.gpsimd.dma_start(wqk1[:], wqk1_e[:, :, :])
            nc.gpsimd.dma_start(wqk2[:], wqk2_e[:, :, :])
            nc.scalar.dma_start(fcon[:], fcon_e[:, :])
            wvB = consts.tile([128, CT, DIM], BF)
            nc.scalar.dma_start(wvB[:], wvB_e[:, :, :])
            wproj = consts.tile([128, CT, DIM], BF)
            nc.scalar.dma_start(wproj[:], wproj_e[:, :, :])

            def qkbias(ot):
                return fcon[:, ot:ot + 1]

            def pbias(ot):
                return fcon[:, 8 + ot:9 + ot]

            def maskb(mt):
                return fcon[:, 12 + mt:13 + mt]

            outT = outp.tile([128, CT, N], BF)
            v_sb = work.tile([128, NT, H, D + 1], BF)
            qk = qkpool.tile([128, OT, N], BF)
            nc.vector.memset(v_sb[:, :, :, D:D + 1], 1.0)
            # early exp-table load on a scratch tile (overlaps the DMA wait)
            nc.scalar.activation(scr2[:], scr[:], AF.Exp, bias=0.0, scale=1.0)

            # ---------- warm-up matmuls (HAM un-throttle during DMA wait) ----------
            wscr = consts.tile([1, 16], F32)
            with tc.tile_pool(name="wps", bufs=1, space="PSUM") as wps:
                wp = wps.tile([128, 512], F32)
                for i in range(9):
                    nc.tensor.matmul(wp[:, :], lhsT=warm[:, 0:128], rhs=warm[:, :],
                                     start=(i == 0), stop=(i == 8))
                # reader so later pools' bank reuse orders after the warm-up
                nc.vector.tensor_copy(wscr[0:1, :], wp[0:1, 0:16])

            with tc.tile_pool(name="lpp", bufs=2, space="PSUM") as lpp, \
                 tc.tile_pool(name="avp", bufs=2, space="PSUM") as avp:

                def qk_group(ot, ncx):
                    items = []
                    box = {}

                    def mk(ct, box=box, ncx=ncx, ot=ot):
                        def mm():
                            if "t" not in box:
                                box["t"] = avp.tile([128, 512], F32, tag="av",
                                                    name=f"qp{ot}_{ncx}")
                            j = QK_ORDER.index(ot)
                            w = wqk1[:, ct, j * 128:(j + 1) * 128] if j < 2 \
                                else wqk2[:, ct, (j - 2) * 128:(j - 1) * 128]
                            nc.tensor.matmul(
                                box["t"][:, :], lhsT=w,
                                rhs=xh[ncx][:, ct, :],
                                start=(ct == 0), stop=(ct == CT - 1))
                        return mm

                    def ev(box=box, ncx=ncx, ot=ot):
                        nc.vector.tensor_scalar_add(
                            qk[:, ot, ncx * 512:(ncx + 1) * 512],
                            box["t"][:, :], qkbias(ot))
                    for ct in range(CT):
                        items.append(mk(ct))
                    items.append(ev)
                    return items

                def qk_items(ot):
                    return qk_group(ot, 0) + qk_group(ot, 1)

                # pair-0 weights in first-ACT critical-path order: the ncx0
                # stream section needs k keys 0:512 (ot4-ncx0) + q ncx0 first
                for it in (qk_group(4, 0) + qk_group(0, 0) +
                           qk_group(4, 1) + qk_group(0, 1)):
                    it()

                def v_items():
                    items = []
                    for mt in range(NT):
                        box = {}

                        def mk(ct, box=box, mt=mt):
                            def mm():
                                if "t" not in box:
                                    box["t"] = avp.tile([128, 512], F32, tag="av",
                                                        name=f"vp{mt}")
                                nc.tensor.matmul(
                                    box["t"][:, :],
                                    lhsT=xh[mt // 4][:, ct, (mt % 4) * 128:
                                                     (mt % 4) * 128 + 128],
                                    rhs=wvB[:, ct, :],
                                    start=(ct == 0), stop=(ct == CT - 1))
                            return mm

                        def ev(box=box, mt=mt):
                            nc.vector.tensor_copy(
                                v_sb[:, mt, :, 0:D],
                                box["t"][:, :].rearrange("p (h d) -> p h d", h=H))
                        for ct in range(CT):
                            items.append(mk(ct))
                        items.append(ev)
                    return items

                # ---------- attention ----------
                st_t = [None, None]

                def av_group(pair, hl, ncx, tail=False):
                    # attn@v accumulation + denominator chain for one head/ncx
                    items = []
                    stp = st_t[pair % 2]
                    h = 2 * pair + hl
                    box = {}

                    def mk(mt, box=box, hl=hl, ncx=ncx, h=h, pair=pair):
                        def mm():
                            if "t" not in box:
                                box["t"] = avp.tile([128, 512], F32, tag="av",
                                                    name=f"ap{h}_{ncx}")
                            p0 = _stpos(pair, mt, hl, ncx)
                            nc.tensor.matmul(
                                box["t"][0:D + 1, :],
                                lhsT=v_sb[:, mt, h, :],
                                rhs=stp[:, p0:p0 + 512],
                                start=(mt == 0), stop=(mt == NT - 1))
                        return mm

                    def chain(box=box, hl=hl, ncx=ncx, pair=pair, h=h, tail=tail):
                        # custom DVE ops drop the input base partition: copy the
                        # den row to partition 0 before the reciprocal
                        dnr = dnpool.tile([1, 512], F32, tag="dnr",
                                          name=f"dnr{h}_{ncx}")
                        if tail:   # ScalarE is idle after its last exp
                            nc.scalar.copy(dnr[0:1, :], box["t"][D:D + 1, :])
                        else:
                            nc.vector.tensor_copy(dnr[0:1, :], box["t"][D:D + 1, :])
                        dn = dnpool.tile([1, 512], F32, tag="dn",
                                         name=f"dn{h}_{ncx}")
                        nc.vector.reciprocal_approx_fast(dn[0:1, :], dnr[0:1, :])
                        bc = bcpool.tile([D, 512], F32, tag="bc",
                                         name=f"bc{h}_{ncx}")
                        nc.gpsimd.partition_broadcast(bc[:, :], dn[0:1, :])
                        nc.vector.tensor_mul(
                            outT[hl * 64:hl * 64 + 64, pair,
                                 ncx * 512:(ncx + 1) * 512],
                            box["t"][0:D, :], bc[:, :])
                    for mt in range(NT):
                        items.append(mk(mt))
                    items.append(chain)
                    return items

                po_sb = popool.tile([128, NC_CH, CT, 512], BF)

                def proj_items(ncx):
                    items = []
                    for ot in range(CT):
                        box = {}

                        def mk(kt, box=box, ot=ot, ncx=ncx):
                            def mm():
                                if "t" not in box:
                                    if ncx == 1 and ot >= 2:
                                        # borrow the (idle in tail) av ring so
                                        # all 4 groups can pre-run kt0..2
                                        box["t"] = avp.tile([128, 512], F32,
                                                            tag="av",
                                                            name=f"pq{ot}")
                                    else:
                                        box["t"] = lpp.tile([128, LPW], F32,
                                                            tag="lp",
                                                            name=f"pp{ot % 2}")
                                nc.tensor.matmul(
                                    box["t"][:, 0:512],
                                    lhsT=wproj[:, kt, ot * 128:(ot + 1) * 128],
                                    rhs=outT[:, kt, ncx * 512:(ncx + 1) * 512],
                                    start=(kt == 0), stop=(kt == CT - 1))
                            return mm

                        def ev(box=box, ot=ot, ncx=ncx):
                            if ncx == 1 and ot % 2 == 0:  # tail: split engines
                                nc.scalar.add(po_sb[:, ncx, ot, :],
                                              box["t"][:, 0:512], pbias(ot))
                            else:
                                nc.vector.tensor_scalar_add(
                                    po_sb[:, ncx, ot, :], box["t"][:, 0:512],
                                    pbias(ot))
                        for kt in range(CT):
                            items.append(mk(kt))
                        items.append(ev)

                    def dma(ncx=ncx):
                        deng = nc.sync if ncx == 0 else nc.gpsimd
                        deng.dma_start(out_e[:, ncx, :, :], po_sb[:, ncx, :, :])
                    items.append(dma)
                    return items

                filler = []
                fi = 0

                def consume(k):
                    nonlocal fi
                    e = min(fi + k, len(filler))
                    while fi < e:
                        filler[fi]()
                        fi += 1

                for pair in range(4):
                    stp = stpool.tile([128, STW], BF, tag="st", name=f"st{pair % 2}")
                    st_t[pair % 2] = stp
                    if pair == 0:
                        # v + next pair's qk weights; later pairs: av(p-1) + qk
                        filler = v_items() + qk_items(1) + qk_items(5)
                        fi = 0
                    elif pair == 1:
                        filler = filler[fi:] + qk_items(2) + qk_items(6)
                        fi = 0
                    elif pair == 2:
                        filler = filler[fi:] + qk_items(3) + qk_items(7)
                        fi = 0
                    elif pair == 3:
                        # phase A: av(2); phase B (from slot 16, after the ncx0
                        # chunks' ACTs are emitted): av(3) ncx0 + proj(ncx0)
                        filler = filler[fi:]
                        fi = 0
                        fillerB = av_group(3, 0, 0) + av_group(3, 1, 0) + \
                            proj_items(0)
                    npts = len(_slots(pair)) // 2
                    if pair == 3:
                        npts = npts // 2   # phase A gets slots 0..15 only
                    per = max(1, (len(filler) - fi + npts - 1) // npts)

                    lpt = {}
                    acted = 0
                    for si, (mt, hl, ncx) in enumerate(_slots(pair)):
                        pos = _stpos(pair, mt, hl, ncx)
                        ti = pos // LPW
                        off = pos - ti * LPW
                        if ti not in lpt:
                            lpt[ti] = lpp.tile([128, LPW], F32, tag="lp",
                                               name=f"lp{pair}_{ti % 2}")
                        pb = hl * 64
                        nc.tensor.matmul(
                            lpt[ti][:, off:off + 512],
                            lhsT=qk[pb:pb + 64, 4 + pair, mt * 128:(mt + 1) * 128],
                            rhs=qk[pb:pb + 64, pair, ncx * 512:(ncx + 1) * 512],
                            start=True, stop=True, tile_position=(pb, 0))
                        if not mask_ones:
                            nc.scalar.activation(
                                stp[:, pos:pos + 512], lpt[ti][:, off:off + 512],
                                AF.Exp, bias=maskb(mt), scale=SCALE)
                            if off + 512 == LPW or pos + 512 == STW:
                                del lpt[ti]
                        else:
                            while acted * LPW + LPW <= pos + 512 or \
                                    pos + 512 == STW:
                                w = min(LPW, STW - acted * LPW)
                                nc.scalar.activation(
                                    stp[:, acted * LPW:acted * LPW + w],
                                    lpt[acted][:, 0:w], AF.Exp,
                                    bias=0.0, scale=SCALE)
                                del lpt[acted]
                                acted += 1
                                if acted * LPW >= STW:
                                    break
                        if pair == 3 and si == 16:
                            consume(len(filler))   # finish phase A
                            filler = fillerB
                            fi = 0
                            per = max(1, (len(filler) + npts - 1) // npts)
                        if si % 2 == 1:
                            consume(per)   # only between hl pairs (PE row pairing)

                    consume(len(filler))
                    if pair < 3:
                        filler = []
                        for ncx in range(NC_CH):
                            for hl in range(2):
                                filler += av_group(pair, hl, ncx)
                        fi = 0

                # ---------- tail: av(3) ncx1 + proj(ncx1) ----------
                avt = av_group(3, 0, 1, tail=True) + av_group(3, 1, 1, tail=True)
                pj = proj_items(1)
                # kt0..2 of all four groups run early (they read pairs 0-2
                # outT only); every kt3 matmul is emitted after both chains.
                # lpp-ring groups (ot0/1) interleave with the first av group;
                # avp-ring groups (ot2/3) only after ap7's tile is allocated.
                pre1 = pj[0:3] + pj[5:8]
                pre2 = pj[10:13] + pj[15:18]
                fin = pj[3:5] + pj[8:10] + pj[13:15] + pj[18:]
                t1 = t2 = 0
                for ai, it in enumerate(avt):
                    it()
                    if ai < 9:
                        if t1 < len(pre1):
                            pre1[t1]()
                            t1 += 1
                    else:
                        if t2 < len(pre2):
                            pre2[t2]()
                            t2 += 1
                while t1 < len(pre1):
                    pre1[t1]()
                    t1 += 1
                while t2 < len(pre2):
                    pre2[t2]()
                    t2 += 1
                for it in fin:
                    it()

    nc.compile()
    return nc


def _prep(inputs):
    x = np.asarray(inputs["x"])
    mask = np.asarray(inputs["mask"])
    qkv_w = np.asarray(inputs["qkv_w"], np.float64)
    CP_U_w = np.asarray(inputs["CP_U_w"], np.float64)
    CP_U_b = np.asarray(inputs["CP_U_b"], np.float64)
    CP_V_w = np.asarray(inputs["CP_V_w"], np.float64)
    CP_V_b = np.asarray(inputs["CP_V_b"], np.float64)
    CP_C = np.asarray(inputs["CP_C"], np.float64)
    CP_att = np.asarray(inputs["CP_attention"], np.float64)
    proj_w = np.asarray(inputs["proj_w"], np.float64)
    proj_b = np.asarray(inputs["proj_b"], np.float64)

    # fold the CP branch (affine in its input) into the dense weights
    F = np.einsum('ijr,rf->fij', CP_C, CP_att)          # (4, R, R)
    UT = CP_U_w.T                                        # (DIM, R)
    VT = CP_V_w.T                                        # (R, DIM)
    A = np.stack([UT @ F[f] @ VT for f in range(4)])     # (4, DIM, DIM)
    c = np.stack([CP_U_b @ F[f] @ VT + CP_V_b for f in range(4)])  # (4, DIM)

    Wqkv = qkv_w.T + np.concatenate([A[0], A[1], A[2]], axis=1)   # (DIM, 3*DIM)
    Wp = proj_w.T + A[3]                                          # (DIM, DIM)
    b_qk = np.concatenate([c[0], c[1]])                           # (2*DIM,)
    b_out = proj_b + c[3] + c[2] @ Wp                             # (DIM,)

    fcon = np.zeros((128, 24), np.float32)
    fcon[:, 0:8] = b_qk.reshape(OT, 128).T
    fcon[:, 8:12] = b_out.reshape(CT, 128).T

    def pmajor(w):
        # (DIM, W) -> (128, CT, W): partition p holds rows {p, 128+p, ...}
        return np.ascontiguousarray(
            w.reshape(CT, 128, w.shape[1]).transpose(1, 0, 2))

    mask_ones = bool(mask.all())
    wqk = pmajor(Wqkv[:, 0:2 * DIM])            # (128, CT, 1024)
    wqk_ord = np.concatenate([wqk[:, :, ot * 128:(ot + 1) * 128]
                              for ot in QK_ORDER], axis=2)
    com = {
        "wqk1": np.ascontiguousarray(wqk_ord[:, :, 0:256]).astype(bf16),
        "wqk2": np.ascontiguousarray(wqk_ord[:, :, 256:1024]).astype(bf16),
        "wvB": pmajor(Wqkv[:, 2 * DIM:3 * DIM]).astype(bf16),
        "wproj": pmajor(Wp).astype(bf16),
    }
    in_maps = []
    for b in range(B):
        m = dict(com)
        xp = pmajor(x[b].T.astype(np.float64))
        m["xA"] = np.ascontiguousarray(xp[:, :, 0:512]).astype(bf16)
        m["xB"] = np.ascontiguousarray(xp[:, :, 512:1024]).astype(bf16)
        fc = fcon.copy()
        if not mask_ones:
            mb = np.where(mask[b], 0.0, -1e30).astype(np.float32)
            fc[:, 12:20] = mb.reshape(NT, 128).T
        m["fcon"] = fc
        in_maps.append(m)
    return in_maps, mask_ones


LAST_EXEC_NS = None


def kernel(**inputs):
    global LAST_EXEC_NS
    in_maps, mask_ones = _prep(inputs)
    key = ("nc", mask_ones)
    if key not in _CACHE:
        _CACHE[key] = _build(mask_ones)
    nc = _CACHE[key]
    res = run_bass_kernel_spmd(nc, in_maps, core_ids=list(range(NCORES)))
    LAST_EXEC_NS = res.exec_time_ns
    outs = []
    for i in range(NCORES):
        ob = np.asarray(res.results[i]["out"], dtype=np.float32)  # (128,2,4,512)
        on = ob.transpose(2, 0, 1, 3).reshape(DIM, N)             # (o, n)
        outs.append(on.T.copy())
    return np.stack(outs).astype(np.float32)


# revision 27
# speedup vs baseline: 1.2683x; 1.0475x over previous
"""Trainium2 Bass kernel for nn_CP_Attention_Action (dense transformer block with
CP-factored low-rank corrections).

Data-parallel over batch B=8 -> one batch per NeuronCore, no collectives.

The CP branch is affine in its input, so it is folded into the dense weights on
the host (f64): W_qkv_eff = qkv_w.T + U.T @ F_f @ V.T blocks, with the q/k
biases applied at evacuation and the v bias folded through softmax (weights sum
to 1) into the proj bias. The device then runs a plain attention block:

  qkT (o,n) via stationary W tiles; v in natural (m,d) layout with a ones
  column so the softmax denominator falls out of attn@v; logits via 64x128
  row-group-paired matmuls (hl0/hl1 issued adjacently -> both stream on PE
  concurrently); exp on ScalarE in 1536-wide batches (all-ones mask -> uniform
  zero bias); denominator: copy+reciprocal on DVE + partition_broadcast on
  GpSimd; attn@v of pair p runs as PE filler during pair p+1; pair 3 uses an
  ncx-major st layout so its ncx0 attn@v half completes mid-pair and only the
  ncx1 half + proj remain in the tail. Host pre-arranges all inputs into
  partition-major layout for fat contiguous DMA descriptors on 4 queues.
"""

import numpy as np
import ml_dtypes

from concourse import bacc
import concourse.mybir as mybir
import concourse.tile as tile
from concourse.bass_utils import run_bass_kernel_spmd

B, N, DIM = 8, 1024, 512
H, D = 8, 64
SCALE = D ** -0.5
NCORES = 8
NC_CH = 2          # n chunks of 512
NT = N // 128      # 8 key tiles
CT = DIM // 128    # 4 c-tiles
OT = 8             # q,k o-tiles
PAIRW = 2 * N      # free elems per key-tile slot in st (2 heads x 1024 q)
STW = NT * PAIRW   # st elems per pair (16384)
LPW = 1536         # activation batch width (3 PSUM banks)
F32 = mybir.dt.float32
BF = mybir.dt.bfloat16
AF = mybir.ActivationFunctionType
bf16 = ml_dtypes.bfloat16

_CACHE = {}


QK_ORDER = (0, 4, 1, 5, 2, 6, 3, 7)


def _stpos(pair, mt, hl, ncx):
    # ncx-major: the first half of each pair's stream only needs the first
    # n-half of q, so pair 0 can start before the second xT half lands
    return ncx * (NT * 1024) + mt * 1024 + hl * 512


def _slots(pair):
    # (mt, hl, ncx) in ascending stream-position order, hl adjacent
    out = []
    for ncx in range(NC_CH):
        for mt in range(NT):
            for hl in range(2):
                out.append((mt, hl, ncx))
    return out


def _build(mask_ones: bool):
    nc = bacc.Bacc()

    xA_e = nc.declare_dram_parameter("xA", [128, CT, 512], BF, isOutput=False)
    xB_e = nc.declare_dram_parameter("xB", [128, CT, 512], BF, isOutput=False)
    wqk1_e = nc.declare_dram_parameter("wqk1", [128, CT, 256], BF, isOutput=False)
    wqk2_e = nc.declare_dram_parameter("wqk2", [128, CT, 768], BF, isOutput=False)
    wvB_e = nc.declare_dram_parameter("wvB", [128, CT, DIM], BF, isOutput=False)
    wproj_e = nc.declare_dram_parameter("wproj", [128, CT, DIM], BF, isOutput=False)
    fcon_e = nc.declare_dram_parameter("fcon", [128, 24], F32, isOutput=False)
    out_e = nc.declare_dram_parameter("out", [128, NC_CH, CT, 512], BF, isOutput=True)

    # fcon layout (f32 columns): 0:8 qk bias per ot | 8:12 proj bias | 12:20 maskb
    with tile.TileContext(nc) as tc:
        with tc.tile_pool(name="consts", bufs=1) as consts, \
             tc.tile_pool(name="qkpool", bufs=1) as qkpool, \
             tc.tile_pool(name="stpool", bufs=2) as stpool, \
             tc.tile_pool(name="work", bufs=1) as work, \
             tc.tile_pool(name="dnpool", bufs=4) as dnpool, \
             tc.tile_pool(name="bcpool", bufs=2) as bcpool, \
             tc.tile_pool(name="outp", bufs=1) as outp, \
             tc.tile_pool(name="popool", bufs=1) as popool:

            # ---------- inputs: 4 DMA queues, partition-major layouts ----------
            warm = consts.tile([128, 512], BF)
            nc.vector.memset(warm[:], 0.0)
            scr = consts.tile([1, 16], BF)
            scr2 = consts.tile([1, 16], BF)
            nc.vector.memset(scr[:], 0.0)
            xh = [consts.tile([128, CT, 512], BF, name=f"xh{i}") for i in range(2)]
            fcon = consts.tile([128, 24], F32)
            wqk1 = consts.tile([128, CT, 256], BF)
            wqk2 = consts.tile([128, CT, 768], BF)
            nc.sync.dma_start(xh[0][:], xA_e[:, :, :])
            nc.sync.dma_start(xh[1][:], xB_e[:, :, :])
            nc.gpsimd.dma_start(wqk1[:], wqk1_e[:, :, :])
            nc.gpsimd.dma_start(wqk2[:], wqk2_e[:, :, :])
            nc.scalar.dma_start(fcon[:], fcon_e[:, :])
            wvB = consts.tile([128, CT, DIM], BF)
            nc.scalar.dma_start(wvB[:], wvB_e[:, :, :])
            wproj = consts.tile([128, CT, DIM], BF)
            nc.scalar.dma_start(wproj[:], wproj_e[:, :, :])

            def qkbias(ot):
                return fcon[:, ot:ot + 1]

            def pbias(ot):
                return fcon[:, 8 + ot:9 + ot]

            def maskb(mt):
                return fcon[:, 12 + mt:13 + mt]

            outT = outp.tile([128, CT, N], BF)
            v_sb = work.tile([128, NT, H, D + 1], BF)
            qk = qkpool.tile([128, OT, N], BF)
            nc.vector.memset(v_sb[:, :, :, D:D + 1], 1.0)
            # early exp-table load on a scratch tile (overlaps the DMA wait)
            nc.scalar.activation(scr2[:], scr[:], AF.Exp, bias=0.0, scale=1.0)

            # ---------- warm-up matmuls (HAM un-throttle during DMA wait) ----------
            wscr = consts.tile([1, 16], F32)
            with tc.tile_pool(name="wps", bufs=1, space="PSUM") as wps:
                wp = wps.tile([128, 512], F32)
                for i in range(7):
                    nc.tensor.matmul(wp[:, :], lhsT=warm[:, 0:128], rhs=warm[:, :],
                                     start=(i == 0), stop=(i == 6))
                # reader so later pools' bank reuse orders after the warm-up
                nc.vector.tensor_copy(wscr[0:1, :], wp[0:1, 0:16])

            with tc.tile_pool(name="lpp", bufs=2, space="PSUM") as lpp, \
                 tc.tile_pool(name="avp", bufs=2, space="PSUM") as avp:

                def qk_group(ot, ncx, ev_eng=None):
                    items = []
                    box = {}

                    def mk(ct, box=box, ncx=ncx, ot=ot):
                        def mm():
                            if "t" not in box:
                                box["t"] = avp.tile([128, 512], F32, tag="av",
                                                    name=f"qp{ot}_{ncx}")
                            j = QK_ORDER.index(ot)
                            w = wqk1[:, ct, j * 128:(j + 1) * 128] if j < 2 \
                                else wqk2[:, ct, (j - 2) * 128:(j - 1) * 128]
                            nc.tensor.matmul(
                                box["t"][:, :], lhsT=w,
                                rhs=xh[ncx][:, ct, :],
                                start=(ct == 0), stop=(ct == CT - 1))
                        return mm

                    def ev(box=box, ncx=ncx, ot=ot, ev_eng=ev_eng):
                        if ev_eng == "scalar":
                            nc.scalar.add(qk[:, ot, ncx * 512:(ncx + 1) * 512],
                                          box["t"][:, :], qkbias(ot))
                        else:
                            nc.vector.tensor_scalar_add(
                                qk[:, ot, ncx * 512:(ncx + 1) * 512],
                                box["t"][:, :], qkbias(ot))
                    for ct in range(CT):
                        items.append(mk(ct))
                    items.append(ev)
                    return items

                def qk_items(ot):
                    return qk_group(ot, 0) + qk_group(ot, 1)

                # pair-0 weights in first-ACT critical-path order: the ncx0
                # stream section needs k keys 0:512 (ot4-ncx0) + q ncx0 first
                for it in (qk_group(4, 0, ev_eng="scalar") + qk_group(0, 0) +
                           qk_group(4, 1, ev_eng="scalar") + qk_group(0, 1)):
                    it()

                def v_items():
                    items = []
                    for mt in range(NT):
                        box = {}

                        def mk(ct, box=box, mt=mt):
                            def mm():
                                if "t" not in box:
                                    box["t"] = avp.tile([128, 512], F32, tag="av",
                                                        name=f"vp{mt}")
                                nc.tensor.matmul(
                                    box["t"][:, :],
                                    lhsT=xh[mt // 4][:, ct, (mt % 4) * 128:
                                                     (mt % 4) * 128 + 128],
                                    rhs=wvB[:, ct, :],
                                    start=(ct == 0), stop=(ct == CT - 1))
                            return mm

                        def ev(box=box, mt=mt):
                            nc.vector.tensor_copy(
                                v_sb[:, mt, :, 0:D],
                                box["t"][:, :].rearrange("p (h d) -> p h d", h=H))
                        for ct in range(CT):
                            items.append(mk(ct))
                        items.append(ev)
                    return items

                # ---------- attention ----------
                st_t = [None, None]

                def av_group(pair, hl, ncx, tail=False):
                    # attn@v accumulation + denominator chain for one head/ncx
                    items = []
                    stp = st_t[pair % 2]
                    h = 2 * pair + hl
                    box = {}

                    def mk(mt, box=box, hl=hl, ncx=ncx, h=h, pair=pair):
                        def mm():
                            if "t" not in box:
                                box["t"] = avp.tile([128, 512], F32, tag="av",
                                                    name=f"ap{h}_{ncx}")
                            p0 = _stpos(pair, mt, hl, ncx)
                            nc.tensor.matmul(
                                box["t"][0:D + 1, :],
                                lhsT=v_sb[:, mt, h, :],
                                rhs=stp[:, p0:p0 + 512],
                                start=(mt == 0), stop=(mt == NT - 1))
                        return mm

                    def chain(box=box, hl=hl, ncx=ncx, pair=pair, h=h, tail=tail):
                        # custom DVE ops drop the input base partition: copy the
                        # den row to partition 0 before the reciprocal
                        dnr = dnpool.tile([1, 512], F32, tag="dnr",
                                          name=f"dnr{h}_{ncx}")
                        if tail:   # ScalarE is idle after its last exp
                            nc.scalar.copy(dnr[0:1, :], box["t"][D:D + 1, :])
                        else:
                            nc.vector.tensor_copy(dnr[0:1, :], box["t"][D:D + 1, :])
                        dn = dnpool.tile([1, 512], F32, tag="dn",
                                         name=f"dn{h}_{ncx}")
                        nc.vector.reciprocal_approx_fast(dn[0:1, :], dnr[0:1, :])
                        bc = bcpool.tile([D, 512], F32, tag="bc",
                                         name=f"bc{h}_{ncx}")
                        nc.gpsimd.partition_broadcast(bc[:, :], dn[0:1, :])
                        nc.vector.tensor_mul(
                            outT[hl * 64:hl * 64 + 64, pair,
                                 ncx * 512:(ncx + 1) * 512],
                            box["t"][0:D, :], bc[:, :])
                    for mt in range(NT):
                        items.append(mk(mt))
                    items.append(chain)
                    return items

                po_sb = popool.tile([128, NC_CH, CT, 512], BF)

                def proj_items(ncx):
                    items = []
                    for ot in range(CT):
                        box = {}

                        def mk(kt, box=box, ot=ot, ncx=ncx):
                            def mm():
                                if "t" not in box:
                                    if ncx == 1 and ot >= 2:
                                        # borrow the (idle in tail) av ring so
                                        # all 4 groups can pre-run kt0..2
                                        box["t"] = avp.tile([128, 512], F32,
                                                            tag="av",
                                                            name=f"pq{ot}")
                                    else:
                                        box["t"] = lpp.tile([128, LPW], F32,
                                                            tag="lp",
                                                            name=f"pp{ot % 2}")
                                nc.tensor.matmul(
                                    box["t"][:, 0:512],
                                    lhsT=wproj[:, kt, ot * 128:(ot + 1) * 128],
                                    rhs=outT[:, kt, ncx * 512:(ncx + 1) * 512],
                                    start=(kt == 0), stop=(kt == CT - 1))
                            return mm

                        def ev(box=box, ot=ot, ncx=ncx):
                            if ncx == 1 and ot % 2 == 0:  # tail: split engines
                                nc.scalar.add(po_sb[:, ncx, ot, :],
                                              box["t"][:, 0:512], pbias(ot))
                            else:
                                nc.vector.tensor_scalar_add(
                                    po_sb[:, ncx, ot, :], box["t"][:, 0:512],
                                    pbias(ot))
                        for kt in range(CT):
                            items.append(mk(kt))
                        items.append(ev)

                    def dma(ncx=ncx):
                        deng = nc.sync if ncx == 0 else nc.gpsimd
                        deng.dma_start(out_e[:, ncx, :, :], po_sb[:, ncx, :, :])
                    items.append(dma)
                    return items

                filler = []
                fi = 0

                def consume(k):
                    nonlocal fi
                    e = min(fi + k, len(filler))
                    while fi < e:
                        filler[fi]()
                        fi += 1

                for pair in range(4):
                    stp = stpool.tile([128, STW], BF, tag="st", name=f"st{pair % 2}")
                    st_t[pair % 2] = stp
                    if pair == 0:
                        filler = v_items() + qk_items(1) + qk_items(5) + \
                            qk_items(2) + qk_items(6)
                        fi = 0
                    elif pair == 1:
                        filler = filler[fi:] + qk_items(3) + qk_items(7)
                        fi = 0
                    elif pair == 2:
                        filler = filler[fi:]
                        fi = 0
                    elif pair == 3:
                        # phase A: av(2); phase B (from slot 16, after the ncx0
                        # chunks' ACTs are emitted): av(3) ncx0 + proj(ncx0)
                        filler = filler[fi:]
                        fi = 0
                        fillerB = av_group(3, 0, 0) + av_group(3, 1, 0) + \
                            proj_items(0)
                    npts = len(_slots(pair)) // 2
                    if pair == 3:
                        npts = npts // 2   # phase A gets slots 0..15 only
                    per = max(1, (len(filler) - fi + npts - 1) // npts)

                    lpt = {}
                    acted = 0
                    for si, (mt, hl, ncx) in enumerate(_slots(pair)):
                        pos = _stpos(pair, mt, hl, ncx)
                        ti = pos // LPW
                        off = pos - ti * LPW
                        if ti not in lpt:
                            lpt[ti] = lpp.tile([128, LPW], F32, tag="lp",
                                               name=f"lp{pair}_{ti % 2}")
                        pb = hl * 64
                        nc.tensor.matmul(
                            lpt[ti][:, off:off + 512],
                            lhsT=qk[pb:pb + 64, 4 + pair, mt * 128:(mt + 1) * 128],
                            rhs=qk[pb:pb + 64, pair, ncx * 512:(ncx + 1) * 512],
                            start=True, stop=True, tile_position=(pb, 0))
                        if not mask_ones:
                            nc.scalar.activation(
                                stp[:, pos:pos + 512], lpt[ti][:, off:off + 512],
                                AF.Exp, bias=maskb(mt), scale=SCALE)
                            if off + 512 == LPW or pos + 512 == STW:
                                del lpt[ti]
                        else:
                            while acted * LPW + LPW <= pos + 512 or \
                                    pos + 512 == STW:
                                w = min(LPW, STW - acted * LPW)
                                nc.scalar.activation(
                                    stp[:, acted * LPW:acted * LPW + w],
                                    lpt[acted][:, 0:w], AF.Exp,
                                    bias=0.0, scale=SCALE)
                                del lpt[acted]
                                acted += 1
                                if acted * LPW >= STW:
                                    break
                        if pair == 3 and si == 16:
                            consume(len(filler))   # finish phase A
                            filler = fillerB
                            fi = 0
                            per = max(1, (len(filler) + npts - 1) // npts)
                        if si % 2 == 1:
                            consume(per)   # only between hl pairs (PE row pairing)

                    consume(len(filler))
                    if pair < 3:
                        filler = []
                        for ncx in range(NC_CH):
                            for hl in range(2):
                                filler += av_group(pair, hl, ncx)
                        fi = 0

                # ---------- tail: av(3) ncx1 + proj(ncx1) ----------
                avt = av_group(3, 0, 1, tail=True) + av_group(3, 1, 1, tail=True)
                pj = proj_items(1)
                # kt0..2 of all four groups run early (they read pairs 0-2
                # outT only); every kt3 matmul is emitted after both chains.
                # lpp-ring groups (ot0/1) interleave with the first av group;
                # avp-ring groups (ot2/3) only after ap7's tile is allocated.
                pre1 = pj[0:3] + pj[5:8]
                pre2 = pj[10:13] + pj[15:18]
                fin = pj[3:5] + pj[8:10] + pj[13:15] + pj[18:]
                t1 = t2 = 0
                for ai, it in enumerate(avt):
                    it()
                    if ai < 9:
                        if t1 < len(pre1):
                            pre1[t1]()
                            t1 += 1
                    else:
                        if t2 < len(pre2):
                            pre2[t2]()
                            t2 += 1
                while t1 < len(pre1):
                    pre1[t1]()
                    t1 += 1
                while t2 < len(pre2):
                    pre2[t2]()
                    t2 += 1
                for it in fin:
                    it()

    nc.compile()
    return nc


def _prep(inputs):
    x = np.asarray(inputs["x"])
    mask = np.asarray(inputs["mask"])
    qkv_w = np.asarray(inputs["qkv_w"], np.float64)
    CP_U_w = np.asarray(inputs["CP_U_w"], np.float64)
    CP_U_b = np.asarray(inputs["CP_U_b"], np.float64)
    CP_V_w = np.asarray(inputs["CP_V_w"], np.float64)
    CP_V_b = np.asarray(inputs["CP_V_b"], np.float64)
    CP_C = np.asarray(inputs["CP_C"], np.float64)
    CP_att = np.asarray(inputs["CP_attention"], np.float64)
    proj_w = np.asarray(inputs["proj_w"], np.float64)
    proj_b = np.asarray(inputs["proj_b"], np.float64)

    # fold the CP branch (affine in its input) into the dense weights
    F = np.einsum('ijr,rf->fij', CP_C, CP_att)          # (4, R, R)
    UT = CP_U_w.T                                        # (DIM, R)
    VT = CP_V_w.T                                        # (R, DIM)
    A = np.stack([UT @ F[f] @ VT for f in range(4)])     # (4, DIM, DIM)
    c = np.stack([CP_U_b @ F[f] @ VT + CP_V_b for f in range(4)])  # (4, DIM)

    Wqkv = qkv_w.T + np.concatenate([A[0], A[1], A[2]], axis=1)   # (DIM, 3*DIM)
    Wp = proj_w.T + A[3]                                          # (DIM, DIM)
    b_qk = np.concatenate([c[0], c[1]])                           # (2*DIM,)
    b_out = proj_b + c[3] + c[2] @ Wp                             # (DIM,)

    fcon = np.zeros((128, 24), np.float32)
    fcon[:, 0:8] = b_qk.reshape(OT, 128).T
    fcon[:, 8:12] = b_out.reshape(CT, 128).T

    def pmajor(w):
        # (DIM, W) -> (128, CT, W): partition p holds rows {p, 128+p, ...}
        return np.ascontiguousarray(
            w.reshape(CT, 128, w.shape[1]).transpose(1, 0, 2))

    mask_ones = bool(mask.all())
    wqk = pmajor(Wqkv[:, 0:2 * DIM])            # (128, CT, 1024)
    wqk_ord = np.concatenate([wqk[:, :, ot * 128:(ot + 1) * 128]
                              for ot in QK_ORDER], axis=2)
    com = {
        "wqk1": np.ascontiguousarray(wqk_ord[:, :, 0:256]).astype(bf16),
        "wqk2": np.ascontiguousarray(wqk_ord[:, :, 256:1024]).astype(bf16),
        "wvB": pmajor(Wqkv[:, 2 * DIM:3 * DIM]).astype(bf16),
        "wproj": pmajor(Wp).astype(bf16),
    }
    in_maps = []
    for b in range(B):
        m = dict(com)
        xp = pmajor(x[b].T.astype(np.float64))
        m["xA"] = np.ascontiguousarray(xp[:, :, 0:512]).astype(bf16)
        m["xB"] = np.ascontiguousarray(xp[:, :, 512:1024]).astype(bf16)
        fc = fcon.copy()
        if not mask_ones:
            mb = np.where(mask[b], 0.0, -1e30).astype(np.float32)
            fc[:, 12:20] = mb.reshape(NT, 128).T
        m["fcon"] = fc
        in_maps.append(m)
    return in_maps, mask_ones


LAST_EXEC_NS = None


def kernel(**inputs):
    global LAST_EXEC_NS
    in_maps, mask_ones = _prep(inputs)
    key = ("nc", mask_ones)
    if key not in _CACHE:
        _CACHE[key] = _build(mask_ones)
    nc = _CACHE[key]
    res = run_bass_kernel_spmd(nc, in_maps, core_ids=list(range(NCORES)))
    LAST_EXEC_NS = res.exec_time_ns
    outs = []
    for i in range(NCORES):
        ob = np.asarray(res.results[i]["out"], dtype=np.float32)  # (128,2,4,512)
        on = ob.transpose(2, 0, 1, 3).reshape(DIM, N)             # (o, n)
        outs.append(on.T.copy())
    return np.stack(outs).astype(np.float32)
